# revision 3
# baseline (speedup 1.0000x reference)
"""Trainium2 Bass kernel for the block-diagonal equivariant linear
(irreps 256x0e + 256x1o + 128x2e, B=32768, D=1664) on 8 NeuronCores.

Strategy: data-parallel over batch (4096 rows/core), weights+bias
replicated. Per 128-row tile: DMA in fp32 -> cast bf16 -> PE transposes
of 13 feature "planes" (strided source APs de-interleave the irrep
components into u-major planes) -> bf16 matmuls vs preloaded block
weights (1/sqrt(mul) folded into the weights host-side) accumulating in
PSUM fp32 -> strided DVE/ACT copies re-interleave planes into the output
layout (+bias on the scalar block) -> DMA out fp32.
"""

import math
import sys

if "/opt/trn_rl_repo" not in sys.path:
    sys.path.insert(0, "/opt/trn_rl_repo")

import ml_dtypes
import numpy as np

import concourse.bass as bass
import concourse.tile as tile
from concourse import bacc, mybir
from concourse.bass_utils import run_bass_kernel_spmd
from concourse.masks import make_identity

# Problem constants (hardcoded; see module docstring).
DIM = 1664
B_TOTAL = 32768
N_CORES = 8
B_CORE = B_TOTAL // N_CORES  # 4096

# (feature_offset, mul, ir_dim) per segment of the flat feature vector.
SEGS = [(0, 256, 1), (256, 256, 3), (1024, 128, 5)]

# 13 transpose "planes": each is 128 contiguous u-values of one (seg, i)
# component. Entries: (src_feature_offset, src_step).
PLANES = []
# Matmuls: (psum_col, N, [(plane_idx, wpk_col), ...]) accumulated over K.
MATMULS = []
# Output copies: (dst_off, dst_step, count, psum_col, seg_idx)
OUTCOPIES = []
# Packed weight columns: [w0a|w0b|w1a|w1b|w2] each [128, N]
_wcol = 0
_pcol = 0  # planar psum column
for _si, (s, mul, d) in enumerate(SEGS):
    nk = mul // 128
    wcols = []
    for c in range(nk):
        PLANES.append((s + d * 128 * c, d))  # i folded in below
        wcols.append(_wcol)
        _wcol += mul
    # one plane-set per component i shares the same weight chunk columns
    for i in range(d):
        planes_i = []
        for c in range(nk):
            planes_i.append((s + i + d * 128 * c, d))
        MATMULS.append((_pcol, mul, [(None, wc) for wc in wcols], planes_i))
        OUTCOPIES.append((s + i, d, mul, _pcol, _si))
        _pcol += mul
# Rebuild PLANES as the flat ordered list implied by MATMULS and assign
# plane indices.
PLANES = []
_MM = []
for pcol, n, wchunks, planes_i in MATMULS:
    chunks = []
    for (_, wc), pl in zip(wchunks, planes_i):
        chunks.append((len(PLANES), wc))
        PLANES.append(pl)
    _MM.append((pcol, n, chunks))
MATMULS = _MM
N_PLANES = len(PLANES)  # 13
assert N_PLANES == 13 and _pcol == DIM

WPK_COLS = _wcol  # 1152


def _host_weights(ws: np.ndarray) -> np.ndarray:
    """Pack per-segment weights, scale folded in, as [128, WPK_COLS] bf16."""
    wpk = np.zeros((128, WPK_COLS), dtype=np.float32)
    col = 0
    off = 0
    for s, mul, d in SEGS:
        w = ws[off : off + mul * mul].reshape(mul, mul) * np.float32(
            1.0 / math.sqrt(mul)
        )
        off += mul * mul
        for c in range(mul // 128):
            wpk[:, col : col + mul] = w[c * 128 : (c + 1) * 128, :]
            col += mul
    return wpk.astype(ml_dtypes.bfloat16)


def build_program(b_core: int = B_CORE, chunk_bt: int = 2, psum_bufs: int = 1):
    """Build + compile the per-core SPMD program. Returns compiled nc."""
    f32 = mybir.dt.float32
    bf16 = mybir.dt.bfloat16
    assert b_core % (128 * chunk_bt) == 0
    n_chunks = b_core // (128 * chunk_bt)

    nc = bacc.Bacc("TRN2", target_bir_lowering=False, debug=False)
    x_ap = nc.dram_tensor("x", [b_core, DIM], f32, kind="ExternalInput").ap()
    wpk_ap = nc.dram_tensor("wpk", [128, WPK_COLS], bf16, kind="ExternalInput").ap()
    bias_ap = nc.dram_tensor("bias", [128, 256], f32, kind="ExternalInput").ap()
    out_ap = nc.dram_tensor("out", [b_core, DIM], f32, kind="ExternalOutput").ap()

    with tile.TileContext(nc) as tc:
        with (
            tc.tile_pool(name="consts", bufs=1) as cpool,
            tc.tile_pool(name="xin", bufs=2) as xin_pool,
            tc.tile_pool(name="x16", bufs=2) as x16_pool,
            tc.tile_pool(name="xT", bufs=2) as xT_pool,
            tc.tile_pool(name="outs", bufs=2) as out_pool,
            tc.tile_pool(name="psT", bufs=2, space="PSUM") as psT_pool,
            tc.tile_pool(name="psO", bufs=psum_bufs, space="PSUM") as psO_pool,
        ):
            wt = cpool.tile([128, WPK_COLS], bf16)
            nc.sync.dma_start(wt[:], wpk_ap[:])
            bias_t = cpool.tile([128, 256], f32)
            nc.sync.dma_start(bias_t[:], bias_ap[:])
            ident = cpool.tile([128, 128], bf16)
            make_identity(nc, ident[:])

            for ch in range(n_chunks):
                rows = 128 * chunk_bt
                src = x_ap[ch * rows : (ch + 1) * rows, :].rearrange(
                    "(r p) f -> p r f", p=128
                )
                xin = xin_pool.tile([128, chunk_bt * DIM], f32)
                nc.sync.dma_start(xin[:].rearrange("p (r f) -> p r f", f=DIM), src)

                x16 = x16_pool.tile([128, chunk_bt * DIM], bf16)
                nc.vector.tensor_copy(x16[:], xin[:])

                outt = out_pool.tile([128, chunk_bt * DIM], f32)
                for r in range(chunk_bt):
                    xv = x16[:, r * DIM : (r + 1) * DIM]
                    ov = outt[:, r * DIM : (r + 1) * DIM]

                    # transpose 13 planes, in groups of 4 per PSUM tile
                    xT = xT_pool.tile([128, N_PLANES * 128], bf16)
                    alt = (ch * chunk_bt + r) % 2
                    for g in range((N_PLANES + 3) // 4):
                        planes = PLANES[g * 4 : g * 4 + 4]
                        psT = psT_pool.tile([128, 512], bf16, tag="psT")
                        for j, (off, step) in enumerate(planes):
                            nc.tensor.transpose(
                                psT[:, j * 128 : (j + 1) * 128],
                                xv[:, off : off + step * 127 + 1 : step],
                                ident[:],
                            )
                        w = 128 * len(planes)
                        dst = xT[:, g * 512 : g * 512 + w]
                        if (g + alt) % 2 == 0:
                            nc.vector.tensor_copy(dst, psT[:, 0:w])
                        else:
                            nc.scalar.copy(dst, psT[:, 0:w])

                    # matmuls into planar psum [128, DIM]
                    ps = psO_pool.tile([128, 2048], f32, tag="psO")
                    for pcol, n, chunks in MATMULS:
                        for k, (pl, wc) in enumerate(chunks):
                            nc.tensor.matmul(
                                ps[:, pcol : pcol + n],
                                xT[:, pl * 128 : (pl + 1) * 128],
                                wt[:, wc : wc + n],
                                start=(k == 0),
                                stop=(k == len(chunks) - 1),
                            )

                    # interleave planar psum back into output layout
                    for oi, (doff, dstep, cnt, pcol, si) in enumerate(OUTCOPIES):
                        dst = ov[:, doff : doff + dstep * (cnt - 1) + 1 : dstep]
                        srcp = ps[:, pcol : pcol + cnt]
                        if si == 0:
                            nc.vector.tensor_add(dst, srcp, bias_t[:])
                        elif si == 1:
                            nc.scalar.copy(dst, srcp)
                        else:
                            if (oi + alt) % 2 == 0:
                                nc.vector.tensor_copy(dst, srcp)
                            else:
                                nc.scalar.copy(dst, srcp)

                dstv = out_ap[ch * rows : (ch + 1) * rows, :].rearrange(
                    "(r p) f -> p r f", p=128
                )
                nc.sync.dma_start(dstv, outt[:].rearrange("p (r f) -> p r f", f=DIM))

    nc.compile()
    return nc


_CACHE: dict = {}


def kernel(ws: np.ndarray, bs: np.ndarray, x: np.ndarray) -> np.ndarray:
    if "nc" not in _CACHE:
        _CACHE["nc"] = build_program()
    nc = _CACHE["nc"]

    wpk = _host_weights(np.asarray(ws, dtype=np.float32))
    bias_t = np.tile(np.asarray(bs, dtype=np.float32)[None, :], (128, 1))
    x = np.ascontiguousarray(x, dtype=np.float32)

    in_maps = [
        {"x": x[i * B_CORE : (i + 1) * B_CORE], "wpk": wpk, "bias": bias_t}
        for i in range(N_CORES)
    ]
    res = run_bass_kernel_spmd(nc, in_maps, list(range(N_CORES)))
    return np.concatenate([r["out"] for r in res.results], axis=0)


# revision 7
# speedup vs baseline: 1.5807x; 1.5807x over previous
"""Trainium2 Bass kernel for the block-diagonal equivariant linear
(irreps 256x0e + 256x1o + 128x2e, B=32768, D=1664) on 8 NeuronCores.

Strategy: data-parallel over batch (4096 rows/core), weights+bias
replicated. Per 128-row tile: DMA in fp32 -> cast bf16 -> PE transposes
of 13 feature "planes" (strided source APs de-interleave the irrep
components into u-major planes) -> bf16 matmuls vs preloaded block
weights (1/sqrt(mul) folded into the weights host-side) accumulating in
PSUM fp32 -> strided DVE/ACT copies re-interleave planes into the output
layout (+bias on the scalar block) -> DMA out fp32.

PE work is software-pipelined with a 1-tile skew (transposes of tile t+1
are emitted between matmul bursts) so copies have a full phase of slack.
"""

import math
import sys

if "/opt/trn_rl_repo" not in sys.path:
    sys.path.insert(0, "/opt/trn_rl_repo")

import ml_dtypes
import numpy as np

import concourse.bass as bass
import concourse.tile as tile
from concourse import bacc, mybir
from concourse.bass_utils import run_bass_kernel_spmd
from concourse.masks import make_identity

# Problem constants (hardcoded; see module docstring).
DIM = 1664
B_TOTAL = 32768
N_CORES = 8
B_CORE = B_TOTAL // N_CORES  # 4096

# (feature_offset, mul, ir_dim) per segment of the flat feature vector.
SEGS = [(0, 256, 1), (256, 256, 3), (1024, 128, 5)]

# Build static op tables.
# PLANES: 13 transpose planes, (src_feature_offset, src_step).
# MATMULS: (out_tile_idx, out_tile_col, N, [(plane_idx, wpk_col), ...]).
# OUTCOPIES: (dst_off, dst_step, count, out_tile_idx, out_tile_col, seg).
PLANES = []
MATMULS = []
OUTCOPIES = []
_wcols = {}
_wcol = 0
for _si, (s, mul, d) in enumerate(SEGS):
    _wcols[_si] = []
    for c in range(mul // 128):
        _wcols[_si].append(_wcol)
        _wcol += mul
WPK_COLS = _wcol  # 1152

# psum out tiles (bank-sized, [128, 512] fp32):
#   tile0: seg0 @0, seg1-i0 @256
#   tile1: seg1-i1 @0, seg1-i2 @256
#   tile2: seg2-i0..3 @128*i
#   tile3: seg2-i4 @0
_PSLOT = {
    (0, 0): (0, 0),
    (1, 0): (0, 256),
    (1, 1): (1, 0),
    (1, 2): (1, 256),
    (2, 0): (2, 0),
    (2, 1): (2, 128),
    (2, 2): (2, 256),
    (2, 3): (2, 384),
    (2, 4): (3, 0),
}
N_PSO = 4
for _si, (s, mul, d) in enumerate(SEGS):
    for i in range(d):
        chunks = []
        for c, wc in enumerate(_wcols[_si]):
            chunks.append((len(PLANES), wc))
            PLANES.append((s + i + d * 128 * c, d))
        ti, tc = _PSLOT[(_si, i)]
        MATMULS.append((ti, tc, mul, chunks))
        OUTCOPIES.append((s + i, d, mul, ti, tc, _si))
N_PLANES = len(PLANES)  # 13
assert N_PLANES == 13

# transpose groups: planes 0..7 -> psT group 0, planes 8..12 -> group 1
TGROUPS = [list(range(8)), list(range(8, 13))]


def _host_weights(ws: np.ndarray) -> np.ndarray:
    """Pack per-segment weights, scale folded in, as [128, WPK_COLS] bf16."""
    wpk = np.zeros((128, WPK_COLS), dtype=np.float32)
    off = 0
    for si, (s, mul, d) in enumerate(SEGS):
        w = ws[off : off + mul * mul].reshape(mul, mul) * np.float32(
            1.0 / math.sqrt(mul)
        )
        off += mul * mul
        for c, col in enumerate(_wcols[si]):
            wpk[:, col : col + mul] = w[c * 128 : (c + 1) * 128, :]
    return wpk.astype(ml_dtypes.bfloat16)


def build_program(b_core: int = B_CORE, chunk_bt: int = 4):
    """Build + compile the per-core SPMD program. Returns compiled nc."""
    f32 = mybir.dt.float32
    bf16 = mybir.dt.bfloat16
    assert b_core % (128 * chunk_bt) == 0
    n_chunks = b_core // (128 * chunk_bt)
    n_bt = n_chunks * chunk_bt

    nc = bacc.Bacc("TRN2", target_bir_lowering=False, debug=False)
    x_ap = nc.dram_tensor("x", [b_core, DIM], f32, kind="ExternalInput").ap()
    wpk_ap = nc.dram_tensor("wpk", [128, WPK_COLS], bf16, kind="ExternalInput").ap()
    bias_ap = nc.dram_tensor("bias", [128, 256], f32, kind="ExternalInput").ap()
    out_ap = nc.dram_tensor("out", [b_core, DIM], f32, kind="ExternalOutput").ap()

    with tile.TileContext(nc) as tc:
        with (
            tc.tile_pool(name="consts", bufs=1) as cpool,
            tc.tile_pool(name="xin", bufs=2) as xin_pool,
            tc.tile_pool(name="x16", bufs=2) as x16_pool,
            tc.tile_pool(name="xT", bufs=3) as xT_pool,
            tc.tile_pool(name="outs", bufs=2) as out_pool,
            tc.tile_pool(name="psT", bufs=3, space="PSUM") as psT_pool,
            tc.tile_pool(name="psO", bufs=5, space="PSUM") as psO_pool,
        ):
            wt = cpool.tile([128, WPK_COLS], bf16)
            nc.sync.dma_start(wt[:], wpk_ap[:])
            bias_t = cpool.tile([128, 256], f32)
            nc.sync.dma_start(bias_t[:], bias_ap[:])
            ident = cpool.tile([128, 128], bf16)
            make_identity(nc, ident[:])

            rows = 128 * chunk_bt
            x16_tiles = {}  # bt -> (x16 tile, col offset)
            xT_tiles = {}  # bt -> xT tile
            ps_tiles = {}  # bt -> [4 psum tiles]
            out_tiles = {}  # bt -> (out tile, col offset)

            def load_chunk(ch):
                src = x_ap[ch * rows : (ch + 1) * rows, :].rearrange(
                    "(r p) f -> p r f", p=128
                )
                xin = xin_pool.tile([128, chunk_bt * DIM], f32)
                nc.sync.dma_start(xin[:].rearrange("p (r f) -> p r f", f=DIM), src)
                x16 = x16_pool.tile([128, chunk_bt * DIM], bf16)
                nc.vector.tensor_copy(x16[:], xin[:])
                outt = out_pool.tile([128, chunk_bt * DIM], f32)
                for r in range(chunk_bt):
                    x16_tiles[ch * chunk_bt + r] = (x16, r * DIM)
                    out_tiles[ch * chunk_bt + r] = (outt, r * DIM)

            def store_chunk(ch):
                outt, _ = out_tiles[ch * chunk_bt]
                dstv = out_ap[ch * rows : (ch + 1) * rows, :].rearrange(
                    "(r p) f -> p r f", p=128
                )
                nc.sync.dma_start(dstv, outt[:].rearrange("p (r f) -> p r f", f=DIM))
                for r in range(chunk_bt):
                    del out_tiles[ch * chunk_bt + r]

            def t_phase(bt):
                """Transposes + psT->xT copies for batch-tile bt."""
                x16, c0 = x16_tiles[bt]
                xT = xT_pool.tile([128, N_PLANES * 128], bf16, tag="xT")
                xT_tiles[bt] = xT
                for g, planes in enumerate(TGROUPS):
                    psT = psT_pool.tile([128, 1024], bf16, tag="psT")
                    for j, pl in enumerate(planes):
                        off, step = PLANES[pl]
                        nc.tensor.transpose(
                            psT[:, j * 128 : (j + 1) * 128],
                            x16[:, c0 + off : c0 + off + step * 127 + 1 : step],
                            ident[:],
                        )
                    w = 128 * len(planes)
                    nc.scalar.copy(xT[:, g * 1024 : g * 1024 + w], psT[:, 0:w])
                del x16_tiles[bt]

            def m_phase(bt):
                """Matmuls + out interleave copies for batch-tile bt."""
                xT = xT_tiles.pop(bt)
                pst = [
                    psO_pool.tile([128, 512], f32, tag="psO", name=f"ps{bt}_{i}")
                    for i in range(N_PSO)
                ]
                for ti, tcol, n, chunks in MATMULS:
                    for k, (pl, wc) in enumerate(chunks):
                        nc.tensor.matmul(
                            pst[ti][:, tcol : tcol + n],
                            xT[:, pl * 128 : (pl + 1) * 128],
                            wt[:, wc : wc + n],
                            start=(k == 0),
                            stop=(k == len(chunks) - 1),
                        )
                ov, c0 = out_tiles[bt]
                for doff, dstep, cnt, ti, tcol, si in OUTCOPIES:
                    dst = ov[:, c0 + doff : c0 + doff + dstep * (cnt - 1) + 1 : dstep]
                    srcp = pst[ti][:, tcol : tcol + cnt]
                    if si == 0:
                        nc.vector.tensor_add(dst, srcp, bias_t[:])
                    elif si == 1:
                        nc.vector.tensor_copy(dst, srcp)
                    else:
                        nc.scalar.copy(dst, srcp)

            # software pipeline with 1-btile skew between T and M phases
            load_chunk(0)
            for bt in range(n_bt + 1):
                if bt < n_bt:
                    t_phase(bt)
                    # prefetch next chunk as soon as we start the last btile
                    # of the current chunk
                    if bt % chunk_bt == chunk_bt - 1:
                        ch = bt // chunk_bt
                        if ch + 1 < n_chunks:
                            load_chunk(ch + 1)
                if bt > 0:
                    m_phase(bt - 1)
                    if bt % chunk_bt == 0:
                        store_chunk(bt // chunk_bt - 1)

    nc.compile()
    return nc


_CACHE: dict = {}


def kernel(ws: np.ndarray, bs: np.ndarray, x: np.ndarray) -> np.ndarray:
    if "nc" not in _CACHE:
        _CACHE["nc"] = build_program()
    nc = _CACHE["nc"]

    wpk = _host_weights(np.asarray(ws, dtype=np.float32))
    bias_t = np.tile(np.asarray(bs, dtype=np.float32)[None, :], (128, 1))
    x = np.ascontiguousarray(x, dtype=np.float32)

    in_maps = [
        {"x": x[i * B_CORE : (i + 1) * B_CORE], "wpk": wpk, "bias": bias_t}
        for i in range(N_CORES)
    ]
    res = run_bass_kernel_spmd(nc, in_maps, list(range(N_CORES)))
    return np.concatenate([r["out"] for r in res.results], axis=0)


# revision 13
# speedup vs baseline: 1.6078x; 1.0172x over previous
"""Trainium2 Bass kernel for the block-diagonal equivariant linear
(irreps 256x0e + 256x1o + 128x2e, B=32768, D=1664) on 8 NeuronCores.

Strategy: data-parallel over batch (4096 rows/core), weights+bias
replicated. Per 128-row tile: DMA in fp32 -> cast bf16 -> PE transposes
of 13 feature "planes" (strided source APs de-interleave the irrep
components into u-major planes) -> bf16 matmuls vs preloaded block
weights (1/sqrt(mul) folded into the weights host-side) accumulating in
PSUM fp32 -> strided DVE/ACT copies re-interleave planes into the output
layout (+bias on the scalar block) -> DMA out fp32.

PE work is software-pipelined with a 1-tile skew (transposes of tile t+1
are emitted between matmul bursts) so copies have a full phase of slack.
"""

import math
import sys

if "/opt/trn_rl_repo" not in sys.path:
    sys.path.insert(0, "/opt/trn_rl_repo")

import ml_dtypes
import numpy as np

import concourse.bass as bass
import concourse.tile as tile
from concourse import bacc, mybir
from concourse.bass_utils import run_bass_kernel_spmd
from concourse.masks import make_identity

# Problem constants (hardcoded; see module docstring).
DIM = 1664
B_TOTAL = 32768
N_CORES = 8
B_CORE = B_TOTAL // N_CORES  # 4096

# (feature_offset, mul, ir_dim) per segment of the flat feature vector.
SEGS = [(0, 256, 1), (256, 256, 3), (1024, 128, 5)]

# Build static op tables.
# PLANES: 13 transpose planes, (src_feature_offset, src_step).
# MATMULS: (out_tile_idx, out_tile_col, N, [(plane_idx, wpk_col), ...]).
# OUTCOPIES: (dst_off, dst_step, count, out_tile_idx, out_tile_col, seg).
PLANES = []
MATMULS = []
OUTCOPIES = []
_wcols = {}
_wcol = 0
for _si, (s, mul, d) in enumerate(SEGS):
    _wcols[_si] = []
    for c in range(mul // 128):
        _wcols[_si].append(_wcol)
        _wcol += mul
WPK_COLS = _wcol  # 1152

# psum out tiles ([128, 1024] fp32, 2 banks each):
#   tile A: seg1-i0 @0, seg1-i1 @256, seg1-i2 @512, seg0 @768
#   tile B: seg2-i0..4 @128*i  (640 cols used)
_PSLOT = {
    (0, 0): (0, 768),
    (1, 0): (0, 0),
    (1, 1): (0, 256),
    (1, 2): (0, 512),
    (2, 0): (1, 0),
    (2, 1): (1, 128),
    (2, 2): (1, 256),
    (2, 3): (1, 384),
    (2, 4): (1, 512),
}
N_PSO = 2
for _si, (s, mul, d) in enumerate(SEGS):
    for i in range(d):
        chunks = []
        for c, wc in enumerate(_wcols[_si]):
            chunks.append((len(PLANES), wc))
            PLANES.append((s + i + d * 128 * c, d))
        ti, tc = _PSLOT[(_si, i)]
        MATMULS.append((ti, tc, mul, chunks))
# emit seg1 first, then seg0 (tile A complete), then seg2 (tile B)
MATMULS = MATMULS[1:4] + MATMULS[0:1] + MATMULS[4:]
N_PLANES = len(PLANES)  # 13
assert N_PLANES == 13

# transpose groups: planes 0..7 -> psT group 0, planes 8..12 -> group 1
TGROUPS = [list(range(8)), list(range(8, 13))]


def _host_weights(ws: np.ndarray) -> np.ndarray:
    """Pack per-segment weights, scale folded in, as [128, WPK_COLS] bf16."""
    wpk = np.zeros((128, WPK_COLS), dtype=np.float32)
    off = 0
    for si, (s, mul, d) in enumerate(SEGS):
        w = ws[off : off + mul * mul].reshape(mul, mul) * np.float32(
            1.0 / math.sqrt(mul)
        )
        off += mul * mul
        for c, col in enumerate(_wcols[si]):
            wpk[:, col : col + mul] = w[c * 128 : (c + 1) * 128, :]
    return wpk.astype(ml_dtypes.bfloat16)


def build_program(b_core: int = B_CORE, chunk_bt: int = 2):
    """Build + compile the per-core SPMD program. Returns compiled nc."""
    f32 = mybir.dt.float32
    bf16 = mybir.dt.bfloat16
    assert b_core % (128 * chunk_bt) == 0
    n_chunks = b_core // (128 * chunk_bt)
    n_bt = n_chunks * chunk_bt

    nc = bacc.Bacc("TRN2", target_bir_lowering=False, debug=False)
    x_ap = nc.dram_tensor("x", [b_core, DIM], f32, kind="ExternalInput").ap()
    wpk_ap = nc.dram_tensor("wpk", [128, WPK_COLS], bf16, kind="ExternalInput").ap()
    bias_ap = nc.dram_tensor("bias", [128, 256], f32, kind="ExternalInput").ap()
    out_ap = nc.dram_tensor("out", [b_core, DIM], f32, kind="ExternalOutput").ap()

    with tile.TileContext(nc) as tc:
        with (
            tc.tile_pool(name="consts", bufs=1) as cpool,
            tc.tile_pool(name="xin", bufs=3) as xin_pool,
            tc.tile_pool(name="x16", bufs=3) as x16_pool,
            tc.tile_pool(name="xT", bufs=3) as xT_pool,
            tc.tile_pool(name="outs", bufs=3) as out_pool,
            tc.tile_pool(name="psT", bufs=2, space="PSUM") as psT_pool,
            tc.tile_pool(name="psO", bufs=3, space="PSUM") as psO_pool,
        ):
            wt = cpool.tile([128, WPK_COLS], bf16)
            nc.sync.dma_start(wt[:], wpk_ap[:])
            bias_t = cpool.tile([128, 256], f32)
            nc.sync.dma_start(bias_t[:], bias_ap[:])
            ident = cpool.tile([128, 128], bf16)
            make_identity(nc, ident[:])

            rows = 128 * chunk_bt
            x16_tiles = {}  # bt -> (x16 tile, col offset)
            xT_tiles = {}  # bt -> xT tile
            ps_tiles = {}  # bt -> [4 psum tiles]
            out_tiles = {}  # bt -> (out tile, col offset)

            def load_chunk(ch):
                src = x_ap[ch * rows : (ch + 1) * rows, :].rearrange(
                    "(r p) f -> p r f", p=128
                )
                xin = xin_pool.tile([128, chunk_bt * DIM], f32)
                nc.sync.dma_start(xin[:].rearrange("p (r f) -> p r f", f=DIM), src)
                x16 = x16_pool.tile([128, chunk_bt * DIM], bf16)
                nc.vector.tensor_copy(x16[:], xin[:])
                outt = out_pool.tile([128, chunk_bt * DIM], f32)
                for r in range(chunk_bt):
                    x16_tiles[ch * chunk_bt + r] = (x16, r * DIM)
                    out_tiles[ch * chunk_bt + r] = (outt, r * DIM)

            def store_chunk(ch):
                outt, _ = out_tiles[ch * chunk_bt]
                dstv = out_ap[ch * rows : (ch + 1) * rows, :].rearrange(
                    "(r p) f -> p r f", p=128
                )
                nc.sync.dma_start(dstv, outt[:].rearrange("p (r f) -> p r f", f=DIM))
                for r in range(chunk_bt):
                    del out_tiles[ch * chunk_bt + r]

            def t_phase(bt):
                """Transposes + psT->xT copies for batch-tile bt."""
                x16, c0 = x16_tiles[bt]
                xT = xT_pool.tile([128, N_PLANES * 128], bf16, tag="xT")
                xT_tiles[bt] = xT
                for g, planes in enumerate(TGROUPS):
                    psT = psT_pool.tile([128, 1024], bf16, tag="psT")
                    for j, pl in enumerate(planes):
                        off, step = PLANES[pl]
                        nc.tensor.transpose(
                            psT[:, j * 128 : (j + 1) * 128],
                            x16[:, c0 + off : c0 + off + step * 127 + 1 : step],
                            ident[:],
                        )
                    w = 128 * len(planes)
                    nc.scalar.copy(xT[:, g * 1024 : g * 1024 + w], psT[:, 0:w])
                del x16_tiles[bt]

            def m_phase(bt):
                """Matmuls + out interleave copies for batch-tile bt."""
                xT = xT_tiles.pop(bt)
                pst = [
                    psO_pool.tile([128, 1024], f32, tag="psO", name=f"ps{bt}_{i}")
                    for i in range(N_PSO)
                ]
                for ti, tcol, n, chunks in MATMULS:
                    for k, (pl, wc) in enumerate(chunks):
                        nc.tensor.matmul(
                            pst[ti][:, tcol : tcol + n],
                            xT[:, pl * 128 : (pl + 1) * 128],
                            wt[:, wc : wc + n],
                            start=(k == 0),
                            stop=(k == len(chunks) - 1),
                        )
                ov, c0 = out_tiles[bt]
                # seg1: one strided-interleave copy (dst col 256+3w+i)
                nc.vector.tensor_copy(
                    ov[:, c0 + 256 : c0 + 1024].rearrange("p (w i) -> p i w", i=3),
                    pst[0][:, 0:768].rearrange("p (i w) -> p i w", w=256),
                )
                # seg0: bias add
                nc.vector.tensor_add(
                    ov[:, c0 : c0 + 256], pst[0][:, 768:1024], bias_t[:]
                )
                # seg2: one strided-interleave copy (dst col 1024+5w+i)
                nc.scalar.copy(
                    ov[:, c0 + 1024 : c0 + 1664].rearrange("p (w i) -> p i w", i=5),
                    pst[1][:, 0:640].rearrange("p (i w) -> p i w", w=128),
                )

            # software pipeline with 1-btile skew between T and M phases;
            # chunk loads are issued a full chunk ahead (xin bufs=3).
            load_chunk(0)
            if n_chunks > 1:
                load_chunk(1)
            for bt in range(n_bt + 1):
                if bt < n_bt:
                    t_phase(bt)
                    if bt % chunk_bt == 0:
                        ch = bt // chunk_bt
                        if ch + 2 < n_chunks:
                            load_chunk(ch + 2)
                if bt > 0:
                    m_phase(bt - 1)
                    if bt % chunk_bt == 0:
                        store_chunk(bt // chunk_bt - 1)

    nc.compile()
    return nc


_CACHE: dict = {}


def kernel(ws: np.ndarray, bs: np.ndarray, x: np.ndarray) -> np.ndarray:
    if "nc" not in _CACHE:
        _CACHE["nc"] = build_program()
    nc = _CACHE["nc"]

    wpk = _host_weights(np.asarray(ws, dtype=np.float32))
    bias_t = np.tile(np.asarray(bs, dtype=np.float32)[None, :], (128, 1))
    x = np.ascontiguousarray(x, dtype=np.float32)

    in_maps = [
        {"x": x[i * B_CORE : (i + 1) * B_CORE], "wpk": wpk, "bias": bias_t}
        for i in range(N_CORES)
    ]
    res = run_bass_kernel_spmd(nc, in_maps, list(range(N_CORES)))
    return np.concatenate([r["out"] for r in res.results], axis=0)


# revision 17
# speedup vs baseline: 1.8735x; 1.1652x over previous
"""Trainium2 Bass kernel for the block-diagonal equivariant linear
(irreps 256x0e + 256x1o + 128x2e, B=32768, D=1664) on 8 NeuronCores.

Strategy: data-parallel over batch (4096 rows/core), weights+bias
replicated. Per 128-row tile: DMA in fp32 -> cast bf16 -> PE transposes
of 13 feature "planes" (strided source APs de-interleave the irrep
components into u-major planes) -> bf16 matmuls vs preloaded block
weights (1/sqrt(mul) folded into the weights host-side) accumulating in
PSUM fp32 -> strided DVE/ACT copies re-interleave planes into the output
layout (+bias on the scalar block) -> DMA out fp32.

PE work is software-pipelined with a 1-tile skew (transposes of tile t+1
are emitted between matmul bursts) so copies have a full phase of slack.
"""

import math
import sys

if "/opt/trn_rl_repo" not in sys.path:
    sys.path.insert(0, "/opt/trn_rl_repo")

import ml_dtypes
import numpy as np

import concourse.bass as bass
import concourse.tile as tile
from concourse import bacc, mybir
from concourse.bass_utils import run_bass_kernel_spmd
from concourse.masks import make_identity

# Problem constants (hardcoded; see module docstring).
DIM = 1664
B_TOTAL = 32768
N_CORES = 8
B_CORE = B_TOTAL // N_CORES  # 4096

# (feature_offset, mul, ir_dim) per segment of the flat feature vector.
SEGS = [(0, 256, 1), (256, 256, 3), (1024, 128, 5)]

# Build static op tables.
# PLANES: 13 transpose planes, (src_feature_offset, src_step).
# MATMULS: (out_tile_idx, out_tile_col, N, [(plane_idx, wpk_col), ...]).
# OUTCOPIES: (dst_off, dst_step, count, out_tile_idx, out_tile_col, seg).
PLANES = []
MATMULS = []
OUTCOPIES = []
_wcols = {}
_wcol = 0
for _si, (s, mul, d) in enumerate(SEGS):
    _wcols[_si] = []
    for c in range(mul // 128):
        _wcols[_si].append(_wcol)
        _wcol += mul
WPK_COLS = _wcol  # 1152

# psum out tiles ([128, 1024] fp32, 2 banks each):
#   tile A: seg1-i0 @0, seg1-i1 @256, seg1-i2 @512, seg0 @768
#   tile B: seg2-i0..4 @128*i  (640 cols used)
_PSLOT = {
    (0, 0): (0, 768),
    (1, 0): (0, 0),
    (1, 1): (0, 256),
    (1, 2): (0, 512),
    (2, 0): (1, 0),
    (2, 1): (1, 128),
    (2, 2): (1, 256),
    (2, 3): (1, 384),
    (2, 4): (1, 512),
}
N_PSO = 2
for _si, (s, mul, d) in enumerate(SEGS):
    for i in range(d):
        chunks = []
        for c, wc in enumerate(_wcols[_si]):
            chunks.append((len(PLANES), wc))
            PLANES.append((s + i + d * 128 * c, d))
        ti, tc = _PSLOT[(_si, i)]
        MATMULS.append((ti, tc, mul, chunks))
# emit seg1 first, then seg0 (tile A complete), then seg2 (tile B)
MATMULS = MATMULS[1:4] + MATMULS[0:1] + MATMULS[4:]
N_PLANES = len(PLANES)  # 13
assert N_PLANES == 13

# transpose groups: planes 0..7 -> psT group 0, planes 8..12 -> group 1
TGROUPS = [list(range(8)), list(range(8, 13))]


def _host_weights(ws: np.ndarray) -> np.ndarray:
    """Pack per-segment weights, scale folded in, as [128, WPK_COLS] bf16."""
    wpk = np.zeros((128, WPK_COLS), dtype=np.float32)
    off = 0
    for si, (s, mul, d) in enumerate(SEGS):
        w = ws[off : off + mul * mul].reshape(mul, mul) * np.float32(
            1.0 / math.sqrt(mul)
        )
        off += mul * mul
        for c, col in enumerate(_wcols[si]):
            wpk[:, col : col + mul] = w[c * 128 : (c + 1) * 128, :]
    return wpk.astype(ml_dtypes.bfloat16)


def build_program(b_core: int = B_CORE, chunk_bt: int = 2):
    """Build + compile the per-core SPMD program. Returns compiled nc."""
    f32 = mybir.dt.float32
    bf16 = mybir.dt.bfloat16
    assert b_core % (128 * chunk_bt) == 0
    n_chunks = b_core // (128 * chunk_bt)
    n_bt = n_chunks * chunk_bt

    nc = bacc.Bacc("TRN2", target_bir_lowering=False, debug=False)
    x_ap = nc.dram_tensor("x", [b_core, DIM], f32, kind="ExternalInput").ap()
    wpk_ap = nc.dram_tensor("wpk", [128, WPK_COLS], bf16, kind="ExternalInput").ap()
    bias_ap = nc.dram_tensor("bias", [128, 256], f32, kind="ExternalInput").ap()
    out_ap = nc.dram_tensor("out", [b_core, DIM], f32, kind="ExternalOutput").ap()

    with tile.TileContext(nc) as tc:
        with (
            tc.tile_pool(name="consts", bufs=1) as cpool,
            tc.tile_pool(name="x16", bufs=3) as x16_pool,
            tc.tile_pool(name="xT", bufs=3) as xT_pool,
            tc.tile_pool(name="outs", bufs=3) as out_pool,
            tc.tile_pool(name="psT", bufs=2, space="PSUM") as psT_pool,
            tc.tile_pool(name="psO", bufs=3, space="PSUM") as psO_pool,
        ):
            wt = cpool.tile([128, WPK_COLS], bf16)
            nc.sync.dma_start(wt[:], wpk_ap[:])
            bias_t = cpool.tile([128, 256], f32)
            nc.sync.dma_start(bias_t[:], bias_ap[:])
            ident = cpool.tile([128, 128], bf16)
            make_identity(nc, ident[:])

            rows = 128 * chunk_bt
            x16_tiles = {}  # bt -> (x16 tile, col offset)
            xT_tiles = {}  # bt -> xT tile
            ps_tiles = {}  # bt -> [4 psum tiles]
            out_tiles = {}  # bt -> (out tile, col offset)

            def load_chunk(ch):
                src = x_ap[ch * rows : (ch + 1) * rows, :].rearrange(
                    "(r p) f -> p r f", p=128
                )
                # SWDGE DMA casts fp32->bf16 in-flight (HWDGE can't cast)
                x16 = x16_pool.tile([128, chunk_bt * DIM], bf16)
                nc.gpsimd.dma_start(x16[:].rearrange("p (r f) -> p r f", f=DIM), src)
                outt = out_pool.tile([128, chunk_bt * DIM], f32)
                for r in range(chunk_bt):
                    x16_tiles[ch * chunk_bt + r] = (x16, r * DIM)
                    out_tiles[ch * chunk_bt + r] = (outt, r * DIM)

            def store_chunk(ch):
                outt, _ = out_tiles[ch * chunk_bt]
                dstv = out_ap[ch * rows : (ch + 1) * rows, :].rearrange(
                    "(r p) f -> p r f", p=128
                )
                nc.sync.dma_start(dstv, outt[:].rearrange("p (r f) -> p r f", f=DIM))
                for r in range(chunk_bt):
                    del out_tiles[ch * chunk_bt + r]

            def t_phase(bt):
                """Transposes + psT->xT copies for batch-tile bt."""
                x16, c0 = x16_tiles[bt]
                xT = xT_pool.tile([128, N_PLANES * 128], bf16, tag="xT")
                xT_tiles[bt] = xT
                for g, planes in enumerate(TGROUPS):
                    psT = psT_pool.tile([128, 1024], bf16, tag="psT")
                    for j, pl in enumerate(planes):
                        off, step = PLANES[pl]
                        nc.tensor.transpose(
                            psT[:, j * 128 : (j + 1) * 128],
                            x16[:, c0 + off : c0 + off + step * 127 + 1 : step],
                            ident[:],
                        )
                    w = 128 * len(planes)
                    if g == 0:
                        nc.scalar.copy(xT[:, g * 1024 : g * 1024 + w], psT[:, 0:w])
                    else:
                        nc.vector.tensor_copy(
                            xT[:, g * 1024 : g * 1024 + w], psT[:, 0:w]
                        )
                del x16_tiles[bt]

            def m_phase(bt):
                """Matmuls + out interleave copies for batch-tile bt."""
                xT = xT_tiles.pop(bt)
                pst = [
                    psO_pool.tile([128, 1024], f32, tag="psO", name=f"ps{bt}_{i}")
                    for i in range(N_PSO)
                ]
                for ti, tcol, n, chunks in MATMULS:
                    for k, (pl, wc) in enumerate(chunks):
                        nc.tensor.matmul(
                            pst[ti][:, tcol : tcol + n],
                            xT[:, pl * 128 : (pl + 1) * 128],
                            wt[:, wc : wc + n],
                            start=(k == 0),
                            stop=(k == len(chunks) - 1),
                        )
                ov, c0 = out_tiles[bt]
                # seg1: one strided-interleave copy (dst col 256+3w+i)
                nc.vector.tensor_copy(
                    ov[:, c0 + 256 : c0 + 1024].rearrange("p (w i) -> p i w", i=3),
                    pst[0][:, 0:768].rearrange("p (i w) -> p i w", w=256),
                )
                # seg0: bias add
                nc.vector.tensor_add(
                    ov[:, c0 : c0 + 256], pst[0][:, 768:1024], bias_t[:]
                )
                # seg2: one strided-interleave copy (dst col 1024+5w+i);
                # unit-stride inner runs on the dst side
                nc.scalar.copy(
                    ov[:, c0 + 1024 : c0 + 1664].rearrange("p (w i) -> p w i", i=5),
                    pst[1][:, 0:640].rearrange("p (i w) -> p w i", w=128),
                )

            # software pipeline with 1-btile skew between T and M phases;
            # chunk loads are issued a full chunk ahead (xin bufs=3).
            load_chunk(0)
            if n_chunks > 1:
                load_chunk(1)
            for bt in range(n_bt + 1):
                if bt < n_bt:
                    t_phase(bt)
                    if bt % chunk_bt == 0:
                        ch = bt // chunk_bt
                        if ch + 2 < n_chunks:
                            load_chunk(ch + 2)
                if bt > 0:
                    m_phase(bt - 1)
                    if bt % chunk_bt == 0:
                        store_chunk(bt // chunk_bt - 1)

    nc.compile()
    return nc


_CACHE: dict = {}


def kernel(ws: np.ndarray, bs: np.ndarray, x: np.ndarray) -> np.ndarray:
    if "nc" not in _CACHE:
        _CACHE["nc"] = build_program()
    nc = _CACHE["nc"]

    wpk = _host_weights(np.asarray(ws, dtype=np.float32))
    bias_t = np.tile(np.asarray(bs, dtype=np.float32)[None, :], (128, 1))
    x = np.ascontiguousarray(x, dtype=np.float32)

    in_maps = [
        {"x": x[i * B_CORE : (i + 1) * B_CORE], "wpk": wpk, "bias": bias_t}
        for i in range(N_CORES)
    ]
    res = run_bass_kernel_spmd(nc, in_maps, list(range(N_CORES)))
    return np.concatenate([r["out"] for r in res.results], axis=0)
